# revision 59
# baseline (speedup 1.0000x reference)
"""AnomalyTransformer forward on 8 trn2 NeuronCores — pure data-parallel over batch.

Feature-major design (HW-validated):
  - residual streams hA (attn input) / hB (FFN input) as per-subgroup
    [128, 400] bf16 tiles; QKV / Wo / FFN / proj contract with K=128
    weight chunks as lhsT; residual adds folded into PE identity-matmuls
  - attention per batch in S^T form with EVERY matmul in full 128x128
    array mode (no row/col tiling, no mode-switch drains):
    psS[m, (par, l)] = K^T Q + mask^T, where K is evacuated into
    head-parity-zeroed tiles Ke/Ko; one exp; softmax denominators via
    [ones|0]/[0|ones] parity-accumulated matmuls; r = exp(-ln(denom)) on
    ScalarE (same act-table set as exp); AV parity-accumulated via
    zero-padded VgPad; normalization fused into the PSUM->SBUF o2
    evacuation (single vector TT)
  - LN: mean broadcast via all-ones/D matmul, centered square, sumsq via
    M=32 ones matmul; rstd magic+Newton on [16,25]-packed stats through a
    DRAM roundtrip; rstd broadcast via K=1 all-ones matmul (NOT gpsimd
    partition_broadcast, which thrashes the Q7 ext-isa library against
    tensor_tensor at ~5us hidden IRAM load per swap)
  - the LN block is software-pipelined 4 deep across groups
    (stats1(g) | sumsq+rstd(g-1) | bcast(g-2) | apply(g-3)) so the PE
    queue never head-of-line blocks on the DVE mean/center/square chain
  - final LN is elided entirely (its input is an LN2 output: mean 0,
    var 1/(1+eps); the affine is folded into the projection host-side);
    the projection fires directly off each LN2 apply inside pass B
  - act-table discipline: exp/ln and gelu phases batched per layer
"""

import os
import sys

import numpy as np

for _p in ("/opt/trn_rl_repo",):
    if _p not in sys.path:
        sys.path.insert(0, _p)

import ml_dtypes
import concourse.bacc as bacc_mod
import concourse.mybir as mybir
from concourse.tile import TileContext
from concourse.bass_utils import run_bass_kernel_spmd

# Act-table steering: the table chooser binds Exp -> exp_and_others and
# Ln -> natural_log (first set containing each fn), which thrashes
# ACT_TABLE_LOADs when the softmax uses r = exp(-ln(denom)). Empty those
# sets (keys stay, so act_func_set_ids stay aligned with act_info.json)
# so both Exp and Ln resolve to natural_log_exp_and_others.
import concourse.hw_specs as _hw_specs


def _patched_gat(arch, _orig=_hw_specs.get_activation_tables):
    t = _orig(arch)
    for k in ("exp_and_others", "natural_log", "exp_and_friends"):
        if k in t:
            t[k] = set()
    return t


_hw_specs.get_activation_tables = _patched_gat
if getattr(bacc_mod, "get_activation_tables", None) is not None:
    bacc_mod.get_activation_tables = _patched_gat

BF16 = ml_dtypes.bfloat16

B, L, CIN, COUT = 256, 100, 38, 38
D, H, E, DFF = 512, 8, 3, 512
DH = D // H
NC_CORES = 8
BL = B // NC_CORES          # 32 batches per core
T = BL * L                  # 3200 tokens per core
GB = 4                      # batches per attention group (= subgroup)
NG = BL // GB               # 8 groups
SGB = 4                     # batches per subgroup (Wo/LN/FFN tile = 400 cols)
NSG = BL // SGB             # 8 subgroups
SGW = SGB * L               # 400
KC = 3 * CIN                # 114 conv contraction rows
NLN = 2 * E + 1             # LN instances

f32 = mybir.dt.float32
f32r = mybir.dt.float32r
fp16 = mybir.dt.float16
bf16 = mybir.dt.bfloat16
i32 = mybir.dt.int32
AF = mybir.ActivationFunctionType
ALU = mybir.AluOpType
AX = mybir.AxisListType

MAGIC_P1 = 0x5F3759DF + 1
KPHASE = int(os.environ.get("KPHASE", "99"))
KFINAL = int(os.environ.get("KFINAL", "1"))
KLN = int(os.environ.get("KLN", "3"))
KATT = int(os.environ.get("KATT", "4"))
KODD = int(os.environ.get("KODD", "1"))   # 1: base-64 operands + row-tile T8
KMSK = int(os.environ.get("KMSK", "0"))   # 1: multiplicative 0/1 mask on DVE


def build_nc(trivial_affine=True, zero_bias=True):
    nc = bacc_mod.Bacc()

    # ---- DRAM parameters ------------------------------------------------
    xaugT = nc.declare_dram_parameter("xaugT", [KC, T], bf16, isOutput=False)
    wcat = nc.declare_dram_parameter("wcat", [KC, D], bf16, isOutput=False)
    petd = nc.declare_dram_parameter("petd", [4, 128, SGW], bf16, isOutput=False)
    wqt = nc.declare_dram_parameter("wqt", [E, 4, 128, D], bf16, isOutput=False)
    wkt = nc.declare_dram_parameter("wkt", [E, 4, 128, D], bf16, isOutput=False)
    wvt = nc.declare_dram_parameter("wvt", [E, 4, 128, D], bf16, isOutput=False)
    wot = nc.declare_dram_parameter("wot", [E, 4, 128, D], bf16, isOutput=False)
    c1wt = nc.declare_dram_parameter("c1wt", [E, 4, 128, D], bf16, isOutput=False)
    c2wt = nc.declare_dram_parameter("c2wt", [E, 4, 128, D], bf16, isOutput=False)
    m01d = nc.declare_dram_parameter("m01d", [L, 8 * L], bf16, isOutput=False)
    maskbd = nc.declare_dram_parameter("maskbd", [L, 4 * L], bf16, isOutput=False)
    identd = nc.declare_dram_parameter("identd", [128, 128], bf16, isOutput=False)
    onesdd = nc.declare_dram_parameter("onesdd", [128, 416], bf16, isOutput=False)
    projt = nc.declare_dram_parameter("projt", [4, 128, COUT], bf16, isOutput=False)
    # bias / affine payloads (used only when the fast flags are off)
    biasd = nc.declare_dram_parameter("biasd", [E, 8, D], bf16, isOutput=False)
    projbd = nc.declare_dram_parameter("projbd", [1, COUT], bf16, isOutput=False)
    affd = nc.declare_dram_parameter("affd", [E, 2, 2, 4, 128], f32, isOutput=False)
    out_d = nc.declare_dram_parameter("out", [COUT, T], f32, isOutput=True)

    statsd = nc.declare_dram_parameter("statsd", [NLN, NSG, SGW], f32,
                                       isOutput=True)
    rowd = nc.declare_dram_parameter("rowd", [NLN, T], f32, isOutput=True)

    with TileContext(nc) as tc:
        with (
            tc.tile_pool(name="const", bufs=1) as cpool,
            tc.tile_pool(name="w", bufs=1) as wpool,
            tc.tile_pool(name="act", bufs=1) as apool,
            tc.tile_pool(name="grp", bufs=2) as gpool,
            tc.tile_pool(name="sc", bufs=3) as spool,
            tc.tile_pool(name="zz", bufs=2) as zpool,
            tc.tile_pool(name="zp", bufs=1) as zppool,
            tc.tile_pool(name="ln", bufs=2) as lpool,
            tc.tile_pool(name="osb", bufs=2) as opool,
            tc.tile_pool(name="ps", bufs=1, space="PSUM") as psum,
        ):
            # ---- embed inputs first (critical path) --------------------
            wcE = cpool.tile([KC, D], bf16, tag="wcE", name="wcE")
            nc.sync.dma_start(out=wcE[:, :], in_=wcat[:, :])
            xaE = cpool.tile([KC, SGW], bf16, tag="xaE", name="xaE")
            nc.sync.dma_start(out=xaE[:, :], in_=xaugT[:, 0:SGW])
            # ---- constants ---------------------------------------------
            idt = cpool.tile([128, 128], bf16, tag="ident", name="ident")
            nc.sync.dma_start(out=idt[:, :], in_=identd[:, :])
            mkb = cpool.tile([L, 4 * L], bf16, tag="mkb", name="mkb")
            nc.sync.dma_start(out=mkb[:, :], in_=maskbd[:, :])
            mk8 = cpool.tile([L, 8 * L], bf16, tag="mk8", name="mk8")
            nc.sync.dma_start(out=mk8[:, :], in_=m01d[:, :])
            onesLd = cpool.tile([128, 416], bf16, tag="onesLd", name="onesLd")
            nc.sync.dma_start(out=onesLd[:, :], in_=onesdd[:, :])
            onesDiv = onesLd[:, 0:128]
            onesP32 = onesLd[:, 128:160]
            onesPadE = onesLd[:, 160:288]   # [ones(64) | zeros(64)] cols
            onesPadO = onesLd[:, 288:416]   # [zeros(64) | ones(64)] cols
            # per-partition 1/0 masks for head-parity-zeroed K evacuation
            zmE = cpool.tile([128, 1], f32, tag="zmE", name="zmE")
            nc.vector.memset(zmE[0:64, :], 1.0)
            nc.vector.memset(zmE[64:128, :], 0.0)
            zmO = cpool.tile([128, 1], f32, tag="zmO", name="zmO")
            nc.vector.memset(zmO[0:64, :], 0.0)
            nc.vector.memset(zmO[64:128, :], 1.0)
            # all-ones [1, 128] f32 row: K=1 broadcast-matmul stationary
            ones1r = cpool.tile([1, 128], f32, tag="ones1r", name="ones1r")
            nc.vector.memset(ones1r[:, :], 1.0)
            if not zero_bias:
                ones1L = cpool.tile([1, L], bf16, tag="ones1L", name="ones1L")
                nc.vector.memset(ones1L[:, :], 1.0)
                onesRow = cpool.tile([1, D], bf16, tag="onesRow",
                                     name="onesRow")
                nc.vector.memset(onesRow[:, :], 1.0)
            pjt = []
            for c in range(4):
                tl = cpool.tile([128, COUT], bf16, tag=f"pjt{c}", name=f"pjt{c}")
                nc.sync.dma_start(out=tl[:, :], in_=projt[c])
                pjt.append(tl)
            pjb = cpool.tile([1, COUT], bf16, tag="pjb", name="pjb")
            nc.sync.dma_start(out=pjb[:, :], in_=projbd[:, :])


            # residual streams, split per subgroup to keep WAR deps local
            hA = [[apool.tile([128, SGW], bf16, tag=f"hA{c}_{s}",
                              name=f"hA{c}_{s}") for s in range(NSG)]
                  for c in range(4)]
            hB = [[apool.tile([128, SGW], bf16, tag=f"hB{c}_{s}",
                              name=f"hB{c}_{s}") for s in range(NSG)]
                  for c in range(4)]

            # round-robin engine pickers
            def tt_eng(i):
                return nc.vector if i % 2 == 0 else nc.gpsimd

            def cp3(i, out, in_):
                if i % 5 < 3:
                    nc.scalar.activation(out, in_, AF.Identity)
                else:
                    nc.vector.tensor_copy(out, in_)

            # ---- LN helpers (split so the sumsq matmul can be deferred a
            # full group behind the mean/center/square DVE chain) ---------
            def evac_z(sg, psZ):
                """Eagerly evacuate Wo/FFN+resid PSUM tiles to SBUF bf16."""
                zg = []
                for c in range(4):
                    t = zpool.tile([128, SGW], bf16, tag=f"zg{c}",
                                   name=f"zg{c}")
                    cp3(c + sg, t[:, :], psZ[c][:, 0:SGW])
                    zg.append(t[:, :])
                return zg

            def ln_stats1(sg, zg, zp_tiles):
                """mean broadcast via all-ones/D matmul, center, square."""
                psM = psum.tile([128, 512], f32, tag="ln", name="psM", bufs=1)
                for c in range(4):
                    nc.tensor.matmul(psM[:, 0:SGW], onesDiv[:, :], zg[c],
                                     start=(c == 0), stop=(c == 3))
                mB = zpool.tile([128, SGW], bf16, tag="mB", name="mB")
                cp3(sg, mB[:, :], psM[:, 0:SGW])
                sq = []
                for c in range(4):
                    tt_eng(c + sg).tensor_sub(zp_tiles[c][:, :], zg[c],
                                              mB[:, :])
                    s = zpool.tile([128, SGW], bf16, tag=f"sq{c}", name=f"sq{c}")
                    tt_eng(c + sg + 1).tensor_mul(s[:, :], zp_tiles[c][:, :],
                                                  zp_tiles[c][:, :])
                    sq.append(s)
                return sq

            def ln_stats2(ln_id, sg, sq):
                psSS = psum.tile([32, 512], f32, tag="ln", name="psSS", bufs=1)
                for c in range(4):
                    nc.tensor.matmul(psSS[:, 0:SGW], onesP32, sq[c][:, :],
                                     start=(c == 0), stop=(c == 3))
                ssb = lpool.tile([1, SGW], f32, tag="ssb", name="ssb", bufs=4)
                cp3(sg, ssb[0:1, 0:SGW], psSS[0:1, 0:SGW])
                nc.sync.dma_start(out=statsd[ln_id, sg].unsqueeze(0),
                                  in_=ssb[0:1, 0:SGW])
                return ssb

            def ln_rstd(ln_id, sg, ssb):
                """rstd for one subgroup: packed [16, 25] magic+Newton
                (pack/unpack via DRAM scratch: SBUF partition dim is
                physical, so a partition-crossing rearrange must go
                through DRAM)."""
                ve = nc.vector
                pk = lpool.tile([16, 25], f32, tag="pk", name="pk", bufs=4)
                nc.sync.dma_start(
                    out=pk[:, :],
                    in_=statsd[ln_id, sg].rearrange("(p f) -> p f", p=16))
                w = lpool.tile([16, 25], f32, tag="lnw", name="lnw", bufs=4)
                y = lpool.tile([16, 25], f32, tag="lny", name="lny", bufs=4)
                t1 = lpool.tile([16, 25], f32, tag="lnt", name="lnt", bufs=4)
                ve.tensor_scalar(w[:, :], pk[:, :], 1.0 / D, 1e-5,
                                 op0=ALU.mult, op1=ALU.add)
                wi = w[:, :].bitcast(i32)
                yi = y[:, :].bitcast(i32)
                ti = t1[:, :].bitcast(i32)
                ve.tensor_scalar(ti, wi, 1, None,
                                 op0=ALU.logical_shift_right)
                ve.tensor_scalar(ti, ti, -1, None, op0=ALU.bitwise_xor)
                ve.tensor_scalar(yi, ti, MAGIC_P1, None, op0=ALU.add)
                for _ in range(2):
                    ve.tensor_mul(t1[:, :], y[:, :], y[:, :])
                    ve.tensor_mul(t1[:, :], t1[:, :], w[:, :])
                    ve.tensor_scalar(t1[:, :], t1[:, :], -0.5, 1.5,
                                     op0=ALU.mult, op1=ALU.add)
                    ve.tensor_mul(y[:, :], y[:, :], t1[:, :])
                nc.sync.dma_start(
                    out=rowd[ln_id, sg * SGW:(sg + 1) * SGW].rearrange(
                        "(p f) -> p f", p=16),
                    in_=y[:, :])
                return y

            def ln_bcast(ln_id, sg, y):
                """rstd row -> [128, SGW] broadcast tile via a K=1 all-ones
                matmul (gpsimd partition_broadcast thrashes the Q7 ext-isa
                library against tensor_tensor: ~5us hidden IRAM load per
                swap)."""
                rw = lpool.tile([1, SGW], f32, tag="rw", name="rw", bufs=4)
                nc.sync.dma_start(out=rw[0:1, :],
                                  in_=rowd[ln_id, sg * SGW:(sg + 1) * SGW]
                                  .unsqueeze(0))
                psB = psum.tile([128, 512], f32, tag="ln", name="psB", bufs=1)
                nc.tensor.matmul(psB[:, 0:SGW], ones1r[0:1, :], rw[0:1, :],
                                 start=True, stop=True)
                rB = lpool.tile([128, SGW], fp16, tag="rB", name="rB", bufs=4)
                cp3(sg, rB[:, :], psB[:, 0:SGW])
                return rB

            def ln_mul(sg, zp_tiles, rB, dst, aff=None):
                """dst[c][sg] = zp[c] * rstd-bcast (* gamma + beta)."""
                for c in range(4):
                    dap = dst[c][sg][:, :]
                    tt_eng(c + sg).tensor_mul(dap, zp_tiles[c][:, :], rB[:, :])
                    if aff is not None:
                        nc.vector.tensor_scalar(dap, dap, aff[0][c][:, 0:1],
                                                aff[1][c][:, 0:1],
                                                op0=ALU.mult, op1=ALU.add)

            def ln_apply(sg, ln_id, zp_tiles, dst, aff=None):
                ln_mul(sg, zp_tiles, ln_bcast(ln_id, sg), dst, aff)

            # ---- embed --------------------------------------------------
            with tc.tile_pool(name="emb", bufs=1) as epool:
                pet = []
                for c in range(4):
                    tl = epool.tile([128, SGW], bf16, tag=f"pet{c}",
                                    name=f"pet{c}")
                    nc.sync.dma_start(out=tl[:, :], in_=petd[c])
                    pet.append(tl)
                wc = wcE
                for sg in range(NSG):
                    cols = slice(sg * SGW, (sg + 1) * SGW)
                    if sg == 0:
                        xa = xaE
                    else:
                        xa = epool.tile([KC, SGW], bf16, tag="xa", name="xa",
                                        bufs=2)
                        nc.sync.dma_start(out=xa[:, :], in_=xaugT[:, cols])
                    for c in range(4):
                        psE = psum.tile([128, 512], f32, tag="mm", name="mm", bufs=3)
                        nc.tensor.matmul(psE[:, 0:SGW],
                                         wc[:, c * 128:(c + 1) * 128],
                                         xa[:, :], start=True, stop=False)
                        nc.tensor.matmul(psE[:, 0:SGW], idt[:, :],
                                         pet[c][:, :], start=False, stop=True)
                        cp3(sg + c, hA[c][sg][:, :], psE[:, 0:SGW])

            # ---- layer weights (per-layer, double-buffered) --------------
            WQ, WK, WV, WO, C1, C2 = {}, {}, {}, {}, {}, {}
            BIAS = {}

            def load_weights(l):
                wop = []
                for p in range(4):
                    tl = wpool.tile([128, D], bf16, tag=f"wop{p}",
                                    name=f"wop{l}{p}")
                    nc.sync.dma_start(out=tl[:, :], in_=wot[l, p])
                    wop.append(tl)
                WO[l] = wop
                for dct, nm, drm in ((WQ, "wq", wqt), (WK, "wk", wkt),
                                     (WV, "wv", wvt),
                                     (C1, "c1", c1wt), (C2, "c2", c2wt)):
                    tiles = []
                    for c in range(4):
                        tl = wpool.tile([128, D], bf16, tag=f"{nm}{c}",
                                        name=f"{nm}{l}{c}")
                        nc.sync.dma_start(out=tl[:, :], in_=drm[l, c])
                        tiles.append(tl)
                    dct[l] = tiles
                if not zero_bias:
                    bt = wpool.tile([8, D], bf16, tag="bias", name=f"bias{l}")
                    nc.sync.dma_start(out=bt[:, :], in_=biasd[l])
                    BIAS[l] = bt
            AFFT = []
            if not trivial_affine:
                for l in range(E):
                    per_ln = []
                    for which in range(2):
                        gs, bs = [], []
                        for c in range(4):
                            g = wpool.tile([128, 1], f32, tag=f"g{l}{which}{c}",
                                           name=f"g{l}{which}{c}")
                            nc.sync.dma_start(out=g[:, :],
                                              in_=affd[l, which, 0, c].unsqueeze(1))
                            bb = wpool.tile([128, 1], f32, tag=f"b{l}{which}{c}",
                                            name=f"b{l}{which}{c}")
                            nc.sync.dma_start(out=bb[:, :],
                                              in_=affd[l, which, 1, c].unsqueeze(1))
                            gs.append(g)
                            bs.append(bb)
                        per_ln.append((gs, bs))
                    AFFT.append(per_ln)

            def bias_row(l, idx):
                # rows: 0 bq,1 bk,2 bv,3 bo,4 c1b,5 c2b
                return BIAS[l][idx:idx + 1, :]

            # ---- per-phase helpers (closures; avoid deep nesting) -------
            def accum_mm(ps, wtiles, rhs_fn, bias_ap):
                for ci in range(4):
                    nc.tensor.matmul(ps, wtiles[ci], rhs_fn(ci),
                                     start=(ci == 0),
                                     stop=(ci == 3 and bias_ap is None))
                if bias_ap is not None:
                    nc.tensor.matmul(ps, bias_ap, onesRow[:, 0:SGW],
                                     start=False, stop=True)

            def do_qkv(l, g):
                """QKV for one group (= subgroup, GB=4 batches). K is
                evacuated twice with per-partition 1/0 scale masks into
                head-parity-zeroed tiles Ke (rows 64-127 = 0) / Ko (rows
                0-63 = 0) so score matmuls stay in full 128x128 mode.
                V lands in VgPad [L, 1024]: per head-pair p the 256-col
                block is [Ve_p(64) | zeros(128) | Vo_p(64)], giving
                [Ve|0] and [0|Vo] lhsT slices for parity-accumulated AV."""
                Qg = [gpool.tile([128, SGW], bf16, tag=f"qg{c}",
                                 name=f"qg{c}") for c in range(4)]
                Ke = [gpool.tile([128, SGW], bf16, tag=f"ke{c}",
                                 name=f"ke{c}") for c in range(4)]
                Ko = [gpool.tile([128, SGW], bf16, tag=f"ko{c}",
                                 name=f"ko{c}") for c in range(4)]
                for co in range(4):
                    for which, dst, wt, brow in ((0, Qg, WQ[l], 0),
                                                 (1, None, WK[l], 1)):
                        ps = psum.tile([128, 512], f32, tag="mm", name="mm",
                                       bufs=3)
                        wts = [wt[ci][:, co * 128:(co + 1) * 128]
                               for ci in range(4)]
                        bias_ap = (None if zero_bias else
                                   BIAS[l][brow:brow + 1,
                                           co * 128:(co + 1) * 128])
                        accum_mm(ps[:, 0:SGW], wts,
                                 lambda ci: hA[ci][g][:, :], bias_ap)
                        if which == 0:
                            cp3(co, dst[co][:, :], ps[:, 0:SGW])
                        else:
                            # zero halves are memset once per pool buffer
                            # (first two groups of layer 0) and never
                            # rewritten; evacuate only the live half.
                            if l == 0 and g < 2:
                                nc.vector.memset(Ke[co][64:128, :], 0.0)
                                nc.vector.memset(Ko[co][0:64, :], 0.0)
                            nc.scalar.activation(Ke[co][0:64, :],
                                                 ps[0:64, 0:SGW],
                                                 AF.Identity)
                            nc.vector.tensor_copy(Ko[co][64:128, :],
                                                  ps[64:128, 0:SGW])
                Vg = [gpool.tile([L, 1024], bf16, tag=f"vg{b}", name=f"vg{b}",
                                 bufs=2) for b in range(GB)]
                for b in range(GB):
                    bl = slice(b * L, (b + 1) * L)
                    psf = psum.tile([128, 512], f32, tag="mm", name="mm", bufs=3)
                    ps = psf[0:L, :]
                    for ci in range(4):
                        nc.tensor.matmul(ps[:, :], hA[ci][g][:, bl],
                                         WV[l][ci],
                                         start=(ci == 0),
                                         stop=(ci == 3 and zero_bias))
                    if not zero_bias:
                        nc.tensor.matmul(ps[:, :], ones1L[:, :],
                                         bias_row(l, 2), start=False, stop=True)
                    vp = Vg[b][:, :].rearrange("t (p x) -> t p x", p=4)
                    sp = ps.rearrange("t (p a d) -> t p a d", p=4, a=2)
                    if l == 0 and g < 2:
                        tt_eng(b).memset(vp[:, :, 64:192], 0.0)
                    cp3(b, vp[:, :, 0:64], sp[:, :, 0, :])
                    cp3(b + 1, vp[:, :, 192:256], sp[:, :, 1, :])
                return Qg, (Ke, Ko), Vg

            def attn_batch(Qg, KgT, Vg, bj, o2):
                """S^T attention, all matmuls in full 128x128 array mode:
                psS[m, (par, p, l)] = K^T Q + mask^T (zero-padded K kills the
                other parity), one exp, denominators via [ones|0]/[0|ones]
                parity-accumulated matmuls, r = exp(-ln(d)) on ScalarE,
                AV parity-accumulated via VgPad, normalization fused into
                the PSUM->SBUF o2 evacuation."""
                Ke, Ko = KgT
                bc = slice(bj * L, (bj + 1) * L)
                psS = psum.tile([L, 1024], f32, tag="S2", name="S2", bufs=1)
                if not KMSK:
                    nc.tensor.matmul(psS[:, 0:4 * L], idt[0:L, 0:L],
                                     mkb[:, :], start=True, stop=False)
                    nc.tensor.matmul(psS[:, 512:512 + 4 * L], idt[0:L, 0:L],
                                     mkb[:, :], start=True, stop=False)
                for co in range(4):
                    cb = co * L
                    nc.tensor.matmul(psS[:, cb:cb + L], Ke[co][:, bc],
                                     Qg[co][:, bc], start=bool(KMSK and co == 0),
                                     stop=(co == 3))
                    nc.tensor.matmul(psS[:, 512 + cb:512 + cb + L],
                                     Ko[co][:, bc],
                                     Qg[co][:, bc], start=bool(KMSK and co == 0),
                                     stop=(co == 3))
                e = spool.tile([L, 8 * L], bf16, tag="e", name="e", bufs=3)
                nc.scalar.activation(
                    e[:, :].rearrange("p (h x) -> p h x", h=2),
                    psS[:, :].rearrange("p (h x) -> p h x", h=2)[:, :, 0:4 * L],
                    AF.Exp)
                if KMSK:
                    # multiplicative 0/1 mask (exact zeros, off the PE)
                    tt_eng(bj).tensor_mul(e[:, :], e[:, :], mk8[:, :])
                if KATT < 2:
                    return
                psD = psum.tile([128, 512], f32, tag="D", name="D", bufs=1)
                nc.tensor.matmul(psD[:, 0:4 * L], onesPadE[0:L, :],
                                 e[:, 0:4 * L], start=True, stop=False)
                nc.tensor.matmul(psD[:, 0:4 * L], onesPadO[0:L, :],
                                 e[:, 4 * L:8 * L], start=False, stop=True)
                # r = exp(-ln(d)) on ScalarE: same act-table set as exp
                # (natural_log_exp_and_others); DVE reciprocal is 8 cyc/elem.
                lnD = spool.tile([128, 4 * L], f32, tag="lnD", name="lnD",
                                 bufs=1)
                nc.scalar.activation(lnD[:, :], psD[:, 0:4 * L], AF.Ln)
                rB = spool.tile([128, 4 * L], fp16, tag="rB", name="rBatt",
                                bufs=2)
                nc.scalar.activation(rB[:, :], lnD[:, :], AF.Exp, scale=-1.0)
                if KATT < 4:
                    return
                psOb = psum.tile([128, 512], f32, tag="Ob", name="Ob", bufs=1)
                for p in range(4):
                    nc.tensor.matmul(
                        psOb[:, p * L:(p + 1) * L],
                        Vg[bj][:, p * 256:p * 256 + 128],
                        e[:, p * L:(p + 1) * L],
                        start=True, stop=False)
                    nc.tensor.matmul(
                        psOb[:, p * L:(p + 1) * L],
                        Vg[bj][:, p * 256 + 128:p * 256 + 256],
                        e[:, 4 * L + p * L:4 * L + (p + 1) * L],
                        start=False, stop=True)
                nc.vector.tensor_mul(
                    o2[:, :].rearrange("q (p w) -> q p w", p=4)
                    [:, :, bj * L:(bj + 1) * L],
                    psOb[:, 0:4 * L].rearrange("q (p m) -> q p m", p=4),
                    rB[:, :].rearrange("q (p m) -> q p m", p=4))

            def do_wo(l, sg, o2):
                psZ = []
                for co in range(4):
                    ps = psum.tile([128, 512], f32, tag="mm", name="mm", bufs=3)
                    for p in range(4):
                        nc.tensor.matmul(ps[:, 0:SGW],
                                         WO[l][p][:, co * 128:(co + 1) * 128],
                                         o2[:, p * SGW:(p + 1) * SGW],
                                         start=(p == 0), stop=False)
                    if not zero_bias:
                        nc.tensor.matmul(ps[:, 0:SGW],
                                         BIAS[l][3:4, co * 128:(co + 1) * 128],
                                         onesRow[:, 0:SGW], start=False,
                                         stop=False)
                    nc.tensor.matmul(ps[:, 0:SGW], idt[:, :],
                                     hA[co][sg][:, :], start=False, stop=True)
                    psZ.append(ps)
                return psZ

            def do_ffn(l, sg):
                cols = slice(sg * SGW, (sg + 1) * SGW)
                Yg = []
                for co in range(4):
                    # borrow the attention-phase PSUM banks (idle in pass B)
                    ps = psum.tile([128, 512], f32,
                                   tag=("S2", "D", "Ob", "mm")[co],
                                   name="ffn1", bufs=(3 if co == 3 else 1))
                    wts = [C1[l][ci][:, co * 128:(co + 1) * 128]
                           for ci in range(4)]
                    bias_ap = (None if zero_bias else
                               BIAS[l][4:5, co * 128:(co + 1) * 128])
                    accum_mm(ps[:, 0:SGW], wts,
                             lambda ci: hB[ci][sg][:, :], bias_ap)
                    yt = zpool.tile([128, SGW], bf16, tag=f"y{co}",
                                    name=f"y{co}")
                    nc.scalar.activation(yt[:, :], ps[:, 0:SGW], AF.Gelu)
                    Yg.append(yt)
                psZ2 = []
                for co in range(4):
                    ps = psum.tile([128, 512], f32, tag="mm", name="mm", bufs=3)
                    for ci in range(4):
                        nc.tensor.matmul(ps[:, 0:SGW],
                                         C2[l][ci][:, co * 128:(co + 1) * 128],
                                         Yg[ci][:, :], start=(ci == 0),
                                         stop=False)
                    if not zero_bias:
                        nc.tensor.matmul(ps[:, 0:SGW],
                                         BIAS[l][5:6, co * 128:(co + 1) * 128],
                                         onesRow[:, 0:SGW], start=False,
                                         stop=False)
                    nc.tensor.matmul(ps[:, 0:SGW], idt[:, :],
                                     hB[co][sg][:, :], start=False, stop=True)
                    psZ2.append(ps)
                return psZ2

            # ---- layers: LN block software-pipelined across groups ------
            # Per group g, emission order is:
            #   qkv/attn/wo/evac (g), ln_stats2+rstd (g-1), ln_stats1 (g),
            #   ln_apply (g-2)
            # so the sumsq matmul for g-1 has a whole group of PE work
            # between it and the DVE chain that feeds it, and the rstd
            # DRAM roundtrip gets a full group of slack before apply.
            def ln_tail(state, ln_id, dst, aff, on_apply=None):
                """Advance the 4-stage deferred-LN pipeline one step:
                sumsq+rstd for g-1, rstd-broadcast for g-2, apply-mul for
                g-3 (records advance through the `state` deque)."""
                for rec in state:
                    rec["age"] += 1
                    if rec["age"] == 1:
                        ssb = ln_stats2(ln_id, rec["sg"], rec["sq"])
                        if KLN >= 2:
                            rec["y"] = ln_rstd(ln_id, rec["sg"], ssb)
                    elif rec["age"] == 2 and KLN >= 3:
                        rec["rB"] = ln_bcast(ln_id, rec["sg"], rec["y"])
                    elif rec["age"] == 3 and KLN >= 3:
                        ln_mul(rec["sg"], rec["zp"], rec["rB"], dst, aff)
                        if on_apply is not None:
                            on_apply(rec["sg"])
                state[:] = [r for r in state if r["age"] < 3]

            def ln_drain(state, ln_id, dst, aff, on_apply=None):
                while state:
                    ln_tail(state, ln_id, dst, aff, on_apply)

            def ln_push(state, sg, zg, tagp):
                zp = [zppool.tile([128, SGW], bf16,
                                  tag=f"zpA_{sg % 4}_{c}",
                                  name=f"{tagp}_{sg}_{c}")
                      for c in range(4)]
                sq = ln_stats1(sg, zg, zp)
                state.append({"sg": sg, "sq": sq, "zp": zp, "rB": None,
                              "age": 0})

            def ln_state():
                return []

            # ---- final LN + projection machinery (driven from inside
            # layer E-1's pass B so the drain overlaps FFN work) ----------
            lnf = 2 * E
            fin_sq, fin_pj, fin_pushed, fin_y = [], [], set(), {}

            def proj_out(sg, zpf, y):
                cols = slice(sg * SGW, (sg + 1) * SGW)
                rB = ln_bcast(lnf, sg, y)
                psf = psum.tile([128, 512], f32,
                                tag=("S2", "D", "Ob", "mm")[sg % 4],
                                name="proj", bufs=(3 if sg % 4 == 3 else 1))
                ps = psf[0:COUT, :]
                for ci in range(4):
                    nc.tensor.matmul(ps[:, 0:SGW], pjt[ci][:, :],
                                     zpf[ci][:, :],
                                     start=(ci == 0),
                                     stop=(ci == 3 and zero_bias))
                if not zero_bias:
                    nc.tensor.matmul(ps[:, 0:SGW], pjb[:, :],
                                     onesRow[:, 0:SGW],
                                     start=False, stop=True)
                osb = opool.tile([COUT, SGW], f32, tag="osb", name="osb")
                nc.vector.tensor_mul(osb[:, :], ps[0:COUT, 0:SGW],
                                     rB[0:COUT, :])
                nc.sync.dma_start(out=out_d[:, cols], in_=osb[:, :])

            def fin_push(sg):
                fin_pushed.add(sg)
                if trivial_affine:
                    # hA is an LN2 output: mean 0, var 1/(1+eps) -- the
                    # final LN is an identity to O(eps), and its affine is
                    # already folded into the projection host-side. Project
                    # straight off the residual stream.
                    cols = slice(sg * SGW, (sg + 1) * SGW)
                    psf = psum.tile([128, 512], f32,
                                    tag=("S2", "D", "Ob", "mm")[sg % 4],
                                    name="proj",
                                    bufs=(3 if sg % 4 == 3 else 1))
                    ps = psf[0:COUT, :]
                    for ci in range(4):
                        nc.tensor.matmul(ps[:, 0:SGW], pjt[ci][:, :],
                                         hA[ci][sg][:, :],
                                         start=(ci == 0), stop=(ci == 3))
                    osb = opool.tile([COUT, SGW], f32, tag="osb", name="osb")
                    cp3(sg, osb[:, :], ps[0:COUT, 0:SGW])
                    nc.sync.dma_start(out=out_d[:, cols], in_=osb[:, :])
                    return
                zpf = [zppool.tile([128, SGW], bf16,
                                   tag=f"zpF_{sg % 4}_{c}",
                                   name=f"zpf_{sg}_{c}")
                       for c in range(4)]
                sqf = ln_stats1(sg, [hA[c][sg][:, :] for c in range(4)], zpf)
                if fin_sq:
                    psg, psqf = fin_sq.pop(0)
                    pssb = ln_stats2(lnf, psg, psqf)
                    fin_y[psg] = ln_rstd(lnf, psg, pssb)
                if len(fin_pj) >= 3:
                    psg, pzpf = fin_pj.pop(0)
                    proj_out(psg, pzpf, fin_y[psg])
                fin_sq.append((sg, sqf))
                fin_pj.append((sg, zpf))

            def fin_drain():
                for psg, psqf in fin_sq:
                    pssb = ln_stats2(lnf, psg, psqf)
                    fin_y[psg] = ln_rstd(lnf, psg, pssb)
                del fin_sq[:]
                for psg, pzpf in fin_pj:
                    proj_out(psg, pzpf, fin_y[psg])
                del fin_pj[:]

            for l in range(E if KPHASE >= 2 else 0):
                ln1, ln2 = 2 * l, 2 * l + 1
                load_weights(l)
                aff1 = None if trivial_affine else AFFT[l][0]
                aff2 = None if trivial_affine else AFFT[l][1]
                # pass A: attention + LN1 for all subgroups (exp act-table)
                stA = ln_state()
                for g in range(NG):
                    Qg, Kg, Vg = do_qkv(l, g)
                    if KPHASE < 3:
                        continue
                    sg = g
                    o2 = gpool.tile([128, 4 * SGW], bf16, tag=f"o2{sg % 2}",
                                    name=f"o2{sg % 2}", bufs=1)
                    for bj in range(SGB):
                        attn_batch(Qg, Kg, Vg, bj, o2)
                    if KPHASE < 4:
                        continue
                    psZ = do_wo(l, sg, o2)
                    if KPHASE < 5:
                        continue
                    zg = evac_z(sg, psZ)
                    ln_tail(stA, ln1, hB, aff1)
                    ln_push(stA, sg, zg, f"zpA{l}")
                if KPHASE >= 5:
                    ln_drain(stA, ln1, hB, aff1)
                if KPHASE < 6:
                    continue
                # pass B: FFN + LN2 for all subgroups (gelu act-table)
                fincb = (fin_push if (l == E - 1 and KFINAL and KLN >= 3)
                         else None)
                stB = ln_state()
                for sg in range(NSG):
                    psZ2 = do_ffn(l, sg)
                    zg2 = evac_z(sg, psZ2)
                    ln_tail(stB, ln2, hA, aff2, on_apply=fincb)
                    ln_push(stB, sg, zg2, f"zpB{l}")
                ln_drain(stB, ln2, hA, aff2, on_apply=fincb)
            # end layers

            # ---- final LN + projection: drain what pass B started -------
            if KPHASE >= 2 and KFINAL:
                for sg in range(NSG):
                    if sg not in fin_pushed:
                        fin_push(sg)
                fin_drain()
            if KPHASE < 2 or not KFINAL:
                # debug: dump embed output
                for sg in range(NSG):
                    cols = slice(sg * SGW, (sg + 1) * SGW)
                    osb = opool.tile([COUT, SGW], f32, tag="osb", name="osb")
                    nc.vector.tensor_copy(osb[:, :], hA[0][sg][0:COUT, :])
                    nc.sync.dma_start(out=out_d[:, cols], in_=osb[:, :])

    nc.compile()
    return nc


# ---------------------------------------------------------------------------
# host side
# ---------------------------------------------------------------------------

def _pos_encoding():
    pos = np.arange(L)[:, None].astype(np.float32)
    div = np.exp(np.arange(0, D, 2).astype(np.float32) * (-np.log(10000.0) / D))
    pe = np.zeros((L, D), dtype=np.float32)
    pe[:, 0::2] = np.sin(pos * div)
    pe[:, 1::2] = np.cos(pos * div)
    return pe


def _chunk4(mT):
    """[D, N] -> [4, 128, N]"""
    return np.ascontiguousarray(mT.reshape(4, 128, -1))


_NC = None
_NC_FLAGS = None


def _get_nc(trivial_affine=True, zero_bias=True):
    global _NC, _NC_FLAGS
    if _NC is None or _NC_FLAGS != (trivial_affine, zero_bias):
        _NC = build_nc(trivial_affine, zero_bias)
        _NC_FLAGS = (trivial_affine, zero_bias)
    return _NC


def is_trivial_affine(inputs):
    i = {k: np.asarray(v) for k, v in inputs.items()}
    return (np.all(i["ln1s"] == 1.0) and np.all(i["ln1b"] == 0.0)
            and np.all(i["ln2s"] == 1.0) and np.all(i["ln2b"] == 0.0))


def is_zero_bias(inputs):
    i = {k: np.asarray(v) for k, v in inputs.items()}
    return all(bool(np.all(i[k] == 0.0))
               for k in ("bq", "bk", "bv", "bo", "c1b", "c2b", "proj_b",
                         "lnfb"))


def prepare_maps(inputs):
    inp = {k: np.asarray(v) for k, v in inputs.items()}
    x = inp["x"].astype(np.float32)
    emb_w = inp["emb_w"].astype(np.float32)
    mask = inp["mask"].astype(np.float32)

    scale = 1.0 / np.sqrt(DH)

    wqt = np.stack([_chunk4(inp["Wq"][l].T * scale) for l in range(E)]).astype(BF16)
    wkt = np.stack([_chunk4(inp["Wk"][l].T) for l in range(E)]).astype(BF16)
    wvt = np.stack([_chunk4(inp["Wv"][l].T) for l in range(E)]).astype(BF16)
    wot = np.stack([_chunk4(inp["Wo"][l].T) for l in range(E)]).astype(BF16)
    c1wt = np.stack([_chunk4(inp["c1w"][l].T) for l in range(E)]).astype(BF16)
    c2wt = np.stack([_chunk4(inp["c2w"][l].T) for l in range(E)]).astype(BF16)

    biasd = np.zeros((E, 8, D), np.float32)
    for l in range(E):
        biasd[l, 0] = inp["bq"][l] * scale
        biasd[l, 1] = inp["bk"][l]
        biasd[l, 2] = inp["bv"][l]
        biasd[l, 3] = inp["bo"][l]
        biasd[l, 4] = inp["c1b"][l]
        biasd[l, 5] = inp["c2b"][l]
    affd = np.zeros((E, 2, 2, 4, 128), np.float32)
    for l in range(E):
        affd[l, 0, 0] = inp["ln1s"][l].reshape(4, 128)
        affd[l, 0, 1] = inp["ln1b"][l].reshape(4, 128)
        affd[l, 1, 0] = inp["ln2s"][l].reshape(4, 128)
        affd[l, 1, 1] = inp["ln2b"][l].reshape(4, 128)

    projw_eff = inp["proj_w"] * inp["lnfs"][None, :]
    projb_eff = inp["proj_b"] + inp["lnfb"] @ inp["proj_w"].T
    projt = np.ascontiguousarray(projw_eff.T.reshape(4, 128, COUT)).astype(BF16)

    pet = np.ascontiguousarray(
        np.tile(_pos_encoding().T.reshape(4, 128, L), (1, 1, SGB))).astype(BF16)
    wcat = np.concatenate([emb_w[:, :, 0].T, emb_w[:, :, 1].T,
                           emb_w[:, :, 2].T], axis=0)
    ident = np.eye(128, dtype=np.float32).astype(BF16)

    onesdd = np.concatenate([
        np.full((128, 128), 1.0 / D, np.float32),
        np.ones((128, 32), np.float32),
        np.concatenate([np.ones((128, 64), np.float32),
                        np.zeros((128, 64), np.float32)], axis=1),
        np.concatenate([np.zeros((128, 64), np.float32),
                        np.ones((128, 64), np.float32)], axis=1)], axis=1)
    shared = dict(
        onesdd=onesdd.astype(BF16),
        wcat=wcat.astype(BF16), petd=pet, wqt=wqt, wkt=wkt, wvt=wvt, wot=wot,
        c1wt=c1wt, c2wt=c2wt, m01d=np.tile(mask.T, (1, 8)).astype(BF16),
        maskbd=np.tile(-30.0 * (1.0 - mask).T, (1, 4)).astype(BF16),
        identd=ident,
        projt=projt, biasd=biasd.astype(BF16),
        projbd=projb_eff.reshape(1, COUT).astype(BF16), affd=affd,
    )

    in_maps = []
    for ci in range(NC_CORES):
        xs = x[ci * BL:(ci + 1) * BL]                      # [32, 100, 38]
        xp = np.concatenate([xs[:, -1:], xs, xs[:, :1]], axis=1)  # [32,102,38]
        feats = [xp[:, w:w + L, :] for w in range(3)]      # each [32,100,38]
        xaug = np.concatenate(feats, axis=2)               # [32,100,114]
        xaugT = np.ascontiguousarray(
            xaug.reshape(T, KC).T).astype(BF16)            # [114, 3200]
        m = dict(shared)
        m["xaugT"] = xaugT
        in_maps.append(m)
    return in_maps


def run(inputs, **kw):
    nc = _get_nc(is_trivial_affine(inputs), is_zero_bias(inputs))
    in_maps = prepare_maps(inputs)
    res = run_bass_kernel_spmd(nc, in_maps, core_ids=list(range(NC_CORES)), **kw)
    outs = []
    for ci in range(NC_CORES):
        o = np.asarray(res.results[ci]["out"], np.float32)  # [38, 3200]
        outs.append(o.T.reshape(BL, L, COUT))
    full = np.concatenate(outs, axis=0)
    return full, res


def kernel(**inputs):
    full, _ = run(inputs)
    return full.astype(np.float32)


def bench(inputs, iters=6):
    """Steady-state wall timing of the sharded jitted executable."""
    import time
    import jax
    from jax.sharding import Mesh, PartitionSpec
    from jax.experimental.shard_map import shard_map
    from concourse import mybir
    from concourse.bass2jax import _bass_exec_p, install_neuronx_cc_hook, partition_id_tensor

    nc = _get_nc(is_trivial_affine(inputs), is_zero_bias(inputs))
    in_maps = prepare_maps(inputs)
    install_neuronx_cc_hook()
    partition_name = nc.partition_id_tensor.name if nc.partition_id_tensor else None
    in_names, out_names, out_avals, zero_outs = [], [], [], []
    for alloc in nc.m.functions[0].allocations:
        if not isinstance(alloc, mybir.MemoryLocationSet):
            continue
        name = alloc.memorylocations[0].name
        if alloc.kind == "ExternalInput":
            if name != partition_name:
                in_names.append(name)
        elif alloc.kind == "ExternalOutput":
            out_names.append(name)
            shape = tuple(alloc.tensor_shape)
            dtype = mybir.dt.np(alloc.dtype)
            out_avals.append(jax.core.ShapedArray(shape, dtype))
            zero_outs.append(np.zeros(shape, dtype))
    n_params = len(in_names)
    n_outs = len(out_avals)
    all_names = list(in_names) + out_names + ([partition_name] if partition_name else [])

    def _body(*args):
        operands = list(args)
        if partition_name is not None:
            operands.append(partition_id_tensor())
        return tuple(_bass_exec_p.bind(
            *operands, out_avals=tuple(out_avals), in_names=tuple(all_names),
            out_names=tuple(out_names), lowering_input_output_aliases=(),
            sim_require_finite=True, sim_require_nnan=True, nc=nc))

    devices = jax.devices()[:NC_CORES]
    mesh = Mesh(np.array(devices), ("core",))
    donate = tuple(range(n_params, n_params + n_outs))
    sharded = jax.jit(
        shard_map(_body, mesh=mesh,
                  in_specs=(PartitionSpec("core"),) * (n_params + n_outs),
                  out_specs=(PartitionSpec("core"),) * n_outs,
                  check_rep=False),
        donate_argnums=donate, keep_unused=True)
    concat_in = [np.concatenate([np.asarray(in_maps[c][n]) for c in range(NC_CORES)], axis=0)
                 for n in in_names]
    dev_in = [jax.device_put(a) for a in concat_in]
    times = []
    out = None
    for it in range(iters):
        zeros = [jax.device_put(np.zeros((NC_CORES * z.shape[0], *z.shape[1:]), z.dtype))
                 for z in zero_outs]
        jax.block_until_ready(zeros)
        t0 = time.perf_counter()
        out = sharded(*dev_in, *zeros)
        jax.block_until_ready(out)
        times.append(time.perf_counter() - t0)
    res = np.asarray(out[0]).reshape(NC_CORES, COUT, T)
    full = np.concatenate([res[c].T.reshape(BL, L, COUT) for c in range(NC_CORES)], axis=0)
    return full, times



# revision 60
# speedup vs baseline: 1.1965x; 1.1965x over previous
"""AnomalyTransformer forward on 8 trn2 NeuronCores — pure data-parallel over batch.

Feature-major design (HW-validated):
  - residual streams hA (attn input) / hB (FFN input) as per-subgroup
    [128, 400] bf16 tiles; QKV / Wo / FFN / proj contract with K=128
    weight chunks as lhsT; residual adds folded into PE identity-matmuls
  - attention per batch in S^T form with EVERY matmul in full 128x128
    array mode (no row/col tiling, no mode-switch drains):
    psS[m, (par, l)] = K^T Q + mask^T, where K is evacuated into
    head-parity-zeroed tiles Ke/Ko; one exp; softmax denominators via
    [ones|0]/[0|ones] parity-accumulated matmuls; r = exp(-ln(denom)) on
    ScalarE (same act-table set as exp); AV parity-accumulated via
    zero-padded VgPad; normalization fused into the PSUM->SBUF o2
    evacuation (single vector TT)
  - LN: mean broadcast via all-ones/D matmul, centered square, sumsq via
    M=32 ones matmul; rstd magic+Newton on [16,25]-packed stats through a
    DRAM roundtrip; rstd broadcast via K=1 all-ones matmul (NOT gpsimd
    partition_broadcast, which thrashes the Q7 ext-isa library against
    tensor_tensor at ~5us hidden IRAM load per swap)
  - the LN block is software-pipelined 4 deep across groups
    (stats1(g) | sumsq+rstd(g-1) | bcast(g-2) | apply(g-3)) so the PE
    queue never head-of-line blocks on the DVE mean/center/square chain
  - final LN is elided entirely (its input is an LN2 output: mean 0,
    var 1/(1+eps); the affine is folded into the projection host-side);
    the projection fires directly off each LN2 apply inside pass B
  - act-table discipline: exp/ln and gelu phases batched per layer
"""

import os
import sys

import numpy as np

for _p in ("/opt/trn_rl_repo",):
    if _p not in sys.path:
        sys.path.insert(0, _p)

import ml_dtypes
import concourse.bacc as bacc_mod
import concourse.mybir as mybir
from concourse.tile import TileContext
from concourse.bass_utils import run_bass_kernel_spmd

# Act-table steering: the table chooser binds Exp -> exp_and_others and
# Ln -> natural_log (first set containing each fn), which thrashes
# ACT_TABLE_LOADs when the softmax uses r = exp(-ln(denom)). Empty those
# sets (keys stay, so act_func_set_ids stay aligned with act_info.json)
# so both Exp and Ln resolve to natural_log_exp_and_others.
import concourse.hw_specs as _hw_specs


def _patched_gat(arch, _orig=_hw_specs.get_activation_tables):
    t = _orig(arch)
    for k in ("exp_and_others", "natural_log", "exp_and_friends"):
        if k in t:
            t[k] = set()
    return t


_hw_specs.get_activation_tables = _patched_gat
if getattr(bacc_mod, "get_activation_tables", None) is not None:
    bacc_mod.get_activation_tables = _patched_gat

BF16 = ml_dtypes.bfloat16

B, L, CIN, COUT = 256, 100, 38, 38
D, H, E, DFF = 512, 8, 3, 512
DH = D // H
NC_CORES = 8
BL = B // NC_CORES          # 32 batches per core
T = BL * L                  # 3200 tokens per core
GB = 4                      # batches per attention group (= subgroup)
NG = BL // GB               # 8 groups
SGB = 4                     # batches per subgroup (Wo/LN/FFN tile = 400 cols)
NSG = BL // SGB             # 8 subgroups
SGW = SGB * L               # 400
KC = 3 * CIN                # 114 conv contraction rows
NLN = 2 * E + 1             # LN instances

f32 = mybir.dt.float32
f32r = mybir.dt.float32r
fp16 = mybir.dt.float16
bf16 = mybir.dt.bfloat16
i32 = mybir.dt.int32
AF = mybir.ActivationFunctionType
ALU = mybir.AluOpType
AX = mybir.AxisListType

MAGIC_P1 = 0x5F3759DF + 1
KPHASE = int(os.environ.get("KPHASE", "99"))
KFINAL = int(os.environ.get("KFINAL", "1"))
KLN = int(os.environ.get("KLN", "3"))
KATT = int(os.environ.get("KATT", "4"))
KODD = int(os.environ.get("KODD", "1"))   # 1: base-64 operands + row-tile T8
KMSK = int(os.environ.get("KMSK", "0"))   # 1: multiplicative 0/1 mask on DVE


def build_nc(trivial_affine=True, zero_bias=True):
    nc = bacc_mod.Bacc()

    # ---- DRAM parameters ------------------------------------------------
    xaugT = nc.declare_dram_parameter("xaugT", [KC, T], bf16, isOutput=False)
    wcat = nc.declare_dram_parameter("wcat", [KC, D], bf16, isOutput=False)
    petd = nc.declare_dram_parameter("petd", [4, 128, SGW], bf16, isOutput=False)
    wqt = nc.declare_dram_parameter("wqt", [E, 4, 128, D], bf16, isOutput=False)
    wkt = nc.declare_dram_parameter("wkt", [E, 4, 128, D], bf16, isOutput=False)
    wvt = nc.declare_dram_parameter("wvt", [E, 4, 128, D], bf16, isOutput=False)
    wot = nc.declare_dram_parameter("wot", [E, 4, 128, D], bf16, isOutput=False)
    c1wt = nc.declare_dram_parameter("c1wt", [E, 4, 128, D], bf16, isOutput=False)
    c2wt = nc.declare_dram_parameter("c2wt", [E, 4, 128, D], bf16, isOutput=False)
    m01d = nc.declare_dram_parameter("m01d", [L, 8 * L], bf16, isOutput=False)
    maskbd = nc.declare_dram_parameter("maskbd", [L, 4 * L], bf16, isOutput=False)
    identd = nc.declare_dram_parameter("identd", [128, 128], bf16, isOutput=False)
    onesdd = nc.declare_dram_parameter("onesdd", [128, 416], bf16, isOutput=False)
    projt = nc.declare_dram_parameter("projt", [4, 128, COUT], bf16, isOutput=False)
    # bias / affine payloads (used only when the fast flags are off)
    biasd = nc.declare_dram_parameter("biasd", [E, 8, D], bf16, isOutput=False)
    projbd = nc.declare_dram_parameter("projbd", [1, COUT], bf16, isOutput=False)
    affd = nc.declare_dram_parameter("affd", [E, 2, 2, 4, 128], f32, isOutput=False)
    out_d = nc.declare_dram_parameter("out", [COUT, T], f32, isOutput=True)

    statsd = nc.declare_dram_parameter("statsd", [NLN, NSG, SGW], f32,
                                       isOutput=True)
    rowd = nc.declare_dram_parameter("rowd", [NLN, T], f32, isOutput=True)

    with TileContext(nc) as tc:
        with (
            tc.tile_pool(name="const", bufs=1) as cpool,
            tc.tile_pool(name="w", bufs=1) as wpool,
            tc.tile_pool(name="act", bufs=1) as apool,
            tc.tile_pool(name="grp", bufs=2) as gpool,
            tc.tile_pool(name="sc", bufs=3) as spool,
            tc.tile_pool(name="zz", bufs=2) as zpool,
            tc.tile_pool(name="zp", bufs=1) as zppool,
            tc.tile_pool(name="ln", bufs=2) as lpool,
            tc.tile_pool(name="osb", bufs=2) as opool,
            tc.tile_pool(name="ps", bufs=1, space="PSUM") as psum,
        ):
            # ---- embed inputs first (critical path) --------------------
            wcE = cpool.tile([KC, D], bf16, tag="wcE", name="wcE")
            nc.sync.dma_start(out=wcE[:, :], in_=wcat[:, :])
            xaE = cpool.tile([KC, SGW], bf16, tag="xaE", name="xaE")
            nc.sync.dma_start(out=xaE[:, :], in_=xaugT[:, 0:SGW])
            # ---- constants ---------------------------------------------
            idt = cpool.tile([128, 128], bf16, tag="ident", name="ident")
            nc.sync.dma_start(out=idt[:, :], in_=identd[:, :])
            mkb = cpool.tile([L, 4 * L], bf16, tag="mkb", name="mkb")
            nc.sync.dma_start(out=mkb[:, :], in_=maskbd[:, :])
            mk8 = cpool.tile([L, 8 * L], bf16, tag="mk8", name="mk8")
            nc.sync.dma_start(out=mk8[:, :], in_=m01d[:, :])
            onesLd = cpool.tile([128, 416], bf16, tag="onesLd", name="onesLd")
            nc.sync.dma_start(out=onesLd[:, :], in_=onesdd[:, :])
            onesDiv = onesLd[:, 0:128]
            onesP32 = onesLd[:, 128:160]
            onesPadE = onesLd[:, 160:288]   # [ones(64) | zeros(64)] cols
            onesPadO = onesLd[:, 288:416]   # [zeros(64) | ones(64)] cols
            # per-partition 1/0 masks for head-parity-zeroed K evacuation
            zmE = cpool.tile([128, 1], f32, tag="zmE", name="zmE")
            nc.vector.memset(zmE[0:64, :], 1.0)
            nc.vector.memset(zmE[64:128, :], 0.0)
            zmO = cpool.tile([128, 1], f32, tag="zmO", name="zmO")
            nc.vector.memset(zmO[0:64, :], 0.0)
            nc.vector.memset(zmO[64:128, :], 1.0)
            # all-ones [1, 128] f32 row: K=1 broadcast-matmul stationary
            ones1r = cpool.tile([1, 128], f32, tag="ones1r", name="ones1r")
            nc.vector.memset(ones1r[:, :], 1.0)
            if not zero_bias:
                ones1L = cpool.tile([1, L], bf16, tag="ones1L", name="ones1L")
                nc.vector.memset(ones1L[:, :], 1.0)
                onesRow = cpool.tile([1, D], bf16, tag="onesRow",
                                     name="onesRow")
                nc.vector.memset(onesRow[:, :], 1.0)
            pjt = []
            for c in range(4):
                tl = cpool.tile([128, COUT], bf16, tag=f"pjt{c}", name=f"pjt{c}")
                nc.sync.dma_start(out=tl[:, :], in_=projt[c])
                pjt.append(tl)
            pjb = cpool.tile([1, COUT], bf16, tag="pjb", name="pjb")
            nc.sync.dma_start(out=pjb[:, :], in_=projbd[:, :])


            # residual streams, split per subgroup to keep WAR deps local
            hA = [[apool.tile([128, SGW], bf16, tag=f"hA{c}_{s}",
                              name=f"hA{c}_{s}") for s in range(NSG)]
                  for c in range(4)]
            hB = [[apool.tile([128, SGW], bf16, tag=f"hB{c}_{s}",
                              name=f"hB{c}_{s}") for s in range(NSG)]
                  for c in range(4)]

            # round-robin engine pickers
            def tt_eng(i):
                return nc.vector if i % 2 == 0 else nc.gpsimd

            def cp3(i, out, in_):
                if i % 5 < 3:
                    nc.scalar.activation(out, in_, AF.Identity)
                else:
                    nc.vector.tensor_copy(out, in_)

            # ---- LN helpers (split so the sumsq matmul can be deferred a
            # full group behind the mean/center/square DVE chain) ---------
            def evac_z(sg, psZ):
                """Eagerly evacuate Wo/FFN+resid PSUM tiles to SBUF bf16."""
                zg = []
                for c in range(4):
                    t = zpool.tile([128, SGW], bf16, tag=f"zg{c}",
                                   name=f"zg{c}")
                    cp3(c + sg, t[:, :], psZ[c][:, 0:SGW])
                    zg.append(t[:, :])
                return zg

            def ln_stats1(sg, zg, zp_tiles):
                """mean broadcast via all-ones/D matmul, center, square."""
                psM = psum.tile([128, 512], f32, tag="ln", name="psM", bufs=1)
                for c in range(4):
                    nc.tensor.matmul(psM[:, 0:SGW], onesDiv[:, :], zg[c],
                                     start=(c == 0), stop=(c == 3))
                mB = zpool.tile([128, SGW], bf16, tag="mB", name="mB")
                cp3(sg, mB[:, :], psM[:, 0:SGW])
                sq = []
                for c in range(4):
                    tt_eng(c + sg).tensor_sub(zp_tiles[c][:, :], zg[c],
                                              mB[:, :])
                    s = zpool.tile([128, SGW], bf16, tag=f"sq{c}", name=f"sq{c}")
                    tt_eng(c + sg + 1).tensor_mul(s[:, :], zp_tiles[c][:, :],
                                                  zp_tiles[c][:, :])
                    sq.append(s)
                return sq

            def ln_stats2(ln_id, sg, sq):
                psSS = psum.tile([32, 512], f32, tag="ln", name="psSS", bufs=1)
                for c in range(4):
                    nc.tensor.matmul(psSS[:, 0:SGW], onesP32, sq[c][:, :],
                                     start=(c == 0), stop=(c == 3))
                ssb = lpool.tile([1, SGW], f32, tag="ssb", name="ssb", bufs=4)
                cp3(sg, ssb[0:1, 0:SGW], psSS[0:1, 0:SGW])
                nc.sync.dma_start(out=statsd[ln_id, sg].unsqueeze(0),
                                  in_=ssb[0:1, 0:SGW])
                return ssb

            def ln_rstd(ln_id, sg, ssb):
                """rstd for one subgroup: packed [16, 25] magic+Newton
                (pack/unpack via DRAM scratch: SBUF partition dim is
                physical, so a partition-crossing rearrange must go
                through DRAM)."""
                ve = nc.vector
                pk = lpool.tile([16, 25], f32, tag="pk", name="pk", bufs=4)
                nc.sync.dma_start(
                    out=pk[:, :],
                    in_=statsd[ln_id, sg].rearrange("(p f) -> p f", p=16))
                w = lpool.tile([16, 25], f32, tag="lnw", name="lnw", bufs=4)
                y = lpool.tile([16, 25], f32, tag="lny", name="lny", bufs=4)
                t1 = lpool.tile([16, 25], f32, tag="lnt", name="lnt", bufs=4)
                ve.tensor_scalar(w[:, :], pk[:, :], 1.0 / D, 1e-5,
                                 op0=ALU.mult, op1=ALU.add)
                wi = w[:, :].bitcast(i32)
                yi = y[:, :].bitcast(i32)
                ti = t1[:, :].bitcast(i32)
                ve.tensor_scalar(ti, wi, 1, None,
                                 op0=ALU.logical_shift_right)
                ve.tensor_scalar(ti, ti, -1, None, op0=ALU.bitwise_xor)
                ve.tensor_scalar(yi, ti, MAGIC_P1, None, op0=ALU.add)
                for _ in range(2):
                    ve.tensor_mul(t1[:, :], y[:, :], y[:, :])
                    ve.tensor_mul(t1[:, :], t1[:, :], w[:, :])
                    ve.tensor_scalar(t1[:, :], t1[:, :], -0.5, 1.5,
                                     op0=ALU.mult, op1=ALU.add)
                    ve.tensor_mul(y[:, :], y[:, :], t1[:, :])
                nc.sync.dma_start(
                    out=rowd[ln_id, sg * SGW:(sg + 1) * SGW].rearrange(
                        "(p f) -> p f", p=16),
                    in_=y[:, :])
                return y

            def ln_bcast(ln_id, sg, y):
                """rstd row -> [128, SGW] broadcast tile via a K=1 all-ones
                matmul (gpsimd partition_broadcast thrashes the Q7 ext-isa
                library against tensor_tensor: ~5us hidden IRAM load per
                swap)."""
                rw = lpool.tile([1, SGW], f32, tag="rw", name="rw", bufs=4)
                nc.sync.dma_start(out=rw[0:1, :],
                                  in_=rowd[ln_id, sg * SGW:(sg + 1) * SGW]
                                  .unsqueeze(0))
                psB = psum.tile([128, 512], f32, tag="ln", name="psB", bufs=1)
                nc.tensor.matmul(psB[:, 0:SGW], ones1r[0:1, :], rw[0:1, :],
                                 start=True, stop=True)
                rB = lpool.tile([128, SGW], fp16, tag="rB", name="rB", bufs=4)
                cp3(sg, rB[:, :], psB[:, 0:SGW])
                return rB

            def ln_mul(sg, zp_tiles, rB, dst, aff=None):
                """dst[c][sg] = zp[c] * rstd-bcast (* gamma + beta)."""
                for c in range(4):
                    dap = dst[c][sg][:, :]
                    tt_eng(c + sg).tensor_mul(dap, zp_tiles[c][:, :], rB[:, :])
                    if aff is not None:
                        nc.vector.tensor_scalar(dap, dap, aff[0][c][:, 0:1],
                                                aff[1][c][:, 0:1],
                                                op0=ALU.mult, op1=ALU.add)

            def ln_apply(sg, ln_id, zp_tiles, dst, aff=None):
                ln_mul(sg, zp_tiles, ln_bcast(ln_id, sg), dst, aff)

            # ---- embed --------------------------------------------------
            with tc.tile_pool(name="emb", bufs=1) as epool:
                pet = []
                for c in range(4):
                    tl = epool.tile([128, SGW], bf16, tag=f"pet{c}",
                                    name=f"pet{c}")
                    nc.sync.dma_start(out=tl[:, :], in_=petd[c])
                    pet.append(tl)
                wc = wcE
                for sg in range(NSG):
                    cols = slice(sg * SGW, (sg + 1) * SGW)
                    if sg == 0:
                        xa = xaE
                    else:
                        xa = epool.tile([KC, SGW], bf16, tag="xa", name="xa",
                                        bufs=2)
                        nc.sync.dma_start(out=xa[:, :], in_=xaugT[:, cols])
                    for c in range(4):
                        psE = psum.tile([128, 512], f32, tag="mm", name="mm", bufs=3)
                        nc.tensor.matmul(psE[:, 0:SGW],
                                         wc[:, c * 128:(c + 1) * 128],
                                         xa[:, :], start=True, stop=False)
                        nc.tensor.matmul(psE[:, 0:SGW], idt[:, :],
                                         pet[c][:, :], start=False, stop=True)
                        cp3(sg + c, hA[c][sg][:, :], psE[:, 0:SGW])

            # ---- layer weights (per-layer, double-buffered) --------------
            WQ, WK, WV, WO, C1, C2 = {}, {}, {}, {}, {}, {}
            BIAS = {}

            def load_weights(l):
                wop = []
                for p in range(4):
                    tl = wpool.tile([128, D], bf16, tag=f"wop{p}",
                                    name=f"wop{l}{p}")
                    nc.sync.dma_start(out=tl[:, :], in_=wot[l, p])
                    wop.append(tl)
                WO[l] = wop
                for dct, nm, drm in ((WQ, "wq", wqt), (WK, "wk", wkt),
                                     (WV, "wv", wvt),
                                     (C1, "c1", c1wt), (C2, "c2", c2wt)):
                    tiles = []
                    for c in range(4):
                        tl = wpool.tile([128, D], bf16, tag=f"{nm}{c}",
                                        name=f"{nm}{l}{c}")
                        nc.sync.dma_start(out=tl[:, :], in_=drm[l, c])
                        tiles.append(tl)
                    dct[l] = tiles
                if not zero_bias:
                    bt = wpool.tile([8, D], bf16, tag="bias", name=f"bias{l}")
                    nc.sync.dma_start(out=bt[:, :], in_=biasd[l])
                    BIAS[l] = bt
            AFFT = []
            if not trivial_affine:
                for l in range(E):
                    per_ln = []
                    for which in range(2):
                        gs, bs = [], []
                        for c in range(4):
                            g = wpool.tile([128, 1], f32, tag=f"g{l}{which}{c}",
                                           name=f"g{l}{which}{c}")
                            nc.sync.dma_start(out=g[:, :],
                                              in_=affd[l, which, 0, c].unsqueeze(1))
                            bb = wpool.tile([128, 1], f32, tag=f"b{l}{which}{c}",
                                            name=f"b{l}{which}{c}")
                            nc.sync.dma_start(out=bb[:, :],
                                              in_=affd[l, which, 1, c].unsqueeze(1))
                            gs.append(g)
                            bs.append(bb)
                        per_ln.append((gs, bs))
                    AFFT.append(per_ln)

            def bias_row(l, idx):
                # rows: 0 bq,1 bk,2 bv,3 bo,4 c1b,5 c2b
                return BIAS[l][idx:idx + 1, :]

            # ---- per-phase helpers (closures; avoid deep nesting) -------
            def accum_mm(ps, wtiles, rhs_fn, bias_ap):
                for ci in range(4):
                    nc.tensor.matmul(ps, wtiles[ci], rhs_fn(ci),
                                     start=(ci == 0),
                                     stop=(ci == 3 and bias_ap is None))
                if bias_ap is not None:
                    nc.tensor.matmul(ps, bias_ap, onesRow[:, 0:SGW],
                                     start=False, stop=True)

            def do_qkv(l, g):
                """QKV for one group (= subgroup, GB=4 batches). K is
                evacuated twice with per-partition 1/0 scale masks into
                head-parity-zeroed tiles Ke (rows 64-127 = 0) / Ko (rows
                0-63 = 0) so score matmuls stay in full 128x128 mode.
                V lands in VgPad [L, 1024]: per head-pair p the 256-col
                block is [Ve_p(64) | zeros(128) | Vo_p(64)], giving
                [Ve|0] and [0|Vo] lhsT slices for parity-accumulated AV."""
                Qg = [gpool.tile([128, SGW], bf16, tag=f"qg{c}",
                                 name=f"qg{c}") for c in range(4)]
                Ke = [gpool.tile([128, SGW], bf16, tag=f"ke{c}",
                                 name=f"ke{c}") for c in range(4)]
                Ko = [gpool.tile([128, SGW], bf16, tag=f"ko{c}",
                                 name=f"ko{c}") for c in range(4)]
                for co in range(4):
                    for which, dst, wt, brow in ((0, Qg, WQ[l], 0),
                                                 (1, None, WK[l], 1)):
                        ps = psum.tile([128, 512], f32, tag="mm", name="mm",
                                       bufs=3)
                        wts = [wt[ci][:, co * 128:(co + 1) * 128]
                               for ci in range(4)]
                        bias_ap = (None if zero_bias else
                                   BIAS[l][brow:brow + 1,
                                           co * 128:(co + 1) * 128])
                        accum_mm(ps[:, 0:SGW], wts,
                                 lambda ci: hA[ci][g][:, :], bias_ap)
                        if which == 0:
                            cp3(co, dst[co][:, :], ps[:, 0:SGW])
                        else:
                            # zero halves are memset once per pool buffer
                            # (first two groups of layer 0) and never
                            # rewritten; evacuate only the live half.
                            if l == 0 and g < 2:
                                nc.vector.memset(Ke[co][64:128, :], 0.0)
                                nc.vector.memset(Ko[co][0:64, :], 0.0)
                            nc.scalar.activation(Ke[co][0:64, :],
                                                 ps[0:64, 0:SGW],
                                                 AF.Identity)
                            nc.vector.tensor_copy(Ko[co][64:128, :],
                                                  ps[64:128, 0:SGW])
                Vg = [gpool.tile([L, 1024], bf16, tag=f"vg{b}", name=f"vg{b}",
                                 bufs=2) for b in range(GB)]
                for b in range(GB):
                    bl = slice(b * L, (b + 1) * L)
                    psf = psum.tile([128, 512], f32, tag="mm", name="mm", bufs=3)
                    ps = psf[0:L, :]
                    for ci in range(4):
                        nc.tensor.matmul(ps[:, :], hA[ci][g][:, bl],
                                         WV[l][ci],
                                         start=(ci == 0),
                                         stop=(ci == 3 and zero_bias))
                    if not zero_bias:
                        nc.tensor.matmul(ps[:, :], ones1L[:, :],
                                         bias_row(l, 2), start=False, stop=True)
                    vp = Vg[b][:, :].rearrange("t (p x) -> t p x", p=4)
                    sp = ps.rearrange("t (p a d) -> t p a d", p=4, a=2)
                    if l == 0 and g < 2:
                        tt_eng(b).memset(vp[:, :, 64:192], 0.0)
                    cp3(b, vp[:, :, 0:64], sp[:, :, 0, :])
                    cp3(b + 1, vp[:, :, 192:256], sp[:, :, 1, :])
                return Qg, (Ke, Ko), Vg

            def attn_batch(Qg, KgT, Vg, bj, o2):
                """S^T attention, all matmuls in full 128x128 array mode:
                psS[m, (par, p, l)] = K^T Q + mask^T (zero-padded K kills the
                other parity), one exp, denominators via [ones|0]/[0|ones]
                parity-accumulated matmuls, r = exp(-ln(d)) on ScalarE,
                AV parity-accumulated via VgPad, normalization fused into
                the PSUM->SBUF o2 evacuation."""
                Ke, Ko = KgT
                bc = slice(bj * L, (bj + 1) * L)
                psS = psum.tile([L, 1024], f32, tag="S2", name="S2", bufs=1)
                if not KMSK:
                    nc.tensor.matmul(psS[:, 0:4 * L], idt[0:L, 0:L],
                                     mkb[:, :], start=True, stop=False)
                    nc.tensor.matmul(psS[:, 512:512 + 4 * L], idt[0:L, 0:L],
                                     mkb[:, :], start=True, stop=False)
                for co in range(4):
                    cb = co * L
                    nc.tensor.matmul(psS[:, cb:cb + L], Ke[co][:, bc],
                                     Qg[co][:, bc], start=bool(KMSK and co == 0),
                                     stop=(co == 3))
                    nc.tensor.matmul(psS[:, 512 + cb:512 + cb + L],
                                     Ko[co][:, bc],
                                     Qg[co][:, bc], start=bool(KMSK and co == 0),
                                     stop=(co == 3))
                e = spool.tile([L, 8 * L], bf16, tag="e", name="e", bufs=2)
                nc.scalar.activation(
                    e[:, :].rearrange("p (h x) -> p h x", h=2),
                    psS[:, :].rearrange("p (h x) -> p h x", h=2)[:, :, 0:4 * L],
                    AF.Exp)
                if KMSK:
                    # multiplicative 0/1 mask (exact zeros, off the PE)
                    tt_eng(bj).tensor_mul(e[:, :], e[:, :], mk8[:, :])
                if KATT < 2:
                    return
                psD = psum.tile([128, 512], f32, tag="D", name="D", bufs=1)
                nc.tensor.matmul(psD[:, 0:4 * L], onesPadE[0:L, :],
                                 e[:, 0:4 * L], start=True, stop=False)
                nc.tensor.matmul(psD[:, 0:4 * L], onesPadO[0:L, :],
                                 e[:, 4 * L:8 * L], start=False, stop=True)
                # r = exp(-ln(d)) on ScalarE: same act-table set as exp
                # (natural_log_exp_and_others); DVE reciprocal is 8 cyc/elem.
                lnD = spool.tile([128, 4 * L], f32, tag="lnD", name="lnD",
                                 bufs=1)
                nc.scalar.activation(lnD[:, :], psD[:, 0:4 * L], AF.Ln)
                rB = spool.tile([128, 4 * L], fp16, tag="rB", name="rBatt",
                                bufs=2)
                nc.scalar.activation(rB[:, :], lnD[:, :], AF.Exp, scale=-1.0)
                if KATT < 4:
                    return
                psOb = psum.tile([128, 512], f32, tag="Ob", name="Ob", bufs=1)
                for p in range(4):
                    nc.tensor.matmul(
                        psOb[:, p * L:(p + 1) * L],
                        Vg[bj][:, p * 256:p * 256 + 128],
                        e[:, p * L:(p + 1) * L],
                        start=True, stop=False)
                    nc.tensor.matmul(
                        psOb[:, p * L:(p + 1) * L],
                        Vg[bj][:, p * 256 + 128:p * 256 + 256],
                        e[:, 4 * L + p * L:4 * L + (p + 1) * L],
                        start=False, stop=True)
                nc.vector.tensor_mul(
                    o2[:, :].rearrange("q (p w) -> q p w", p=4)
                    [:, :, bj * L:(bj + 1) * L],
                    psOb[:, 0:4 * L].rearrange("q (p m) -> q p m", p=4),
                    rB[:, :].rearrange("q (p m) -> q p m", p=4))

            def do_wo(l, sg, o2):
                psZ = []
                for co in range(4):
                    ps = psum.tile([128, 512], f32, tag="mm", name="mm", bufs=3)
                    for p in range(4):
                        nc.tensor.matmul(ps[:, 0:SGW],
                                         WO[l][p][:, co * 128:(co + 1) * 128],
                                         o2[:, p * SGW:(p + 1) * SGW],
                                         start=(p == 0), stop=False)
                    if not zero_bias:
                        nc.tensor.matmul(ps[:, 0:SGW],
                                         BIAS[l][3:4, co * 128:(co + 1) * 128],
                                         onesRow[:, 0:SGW], start=False,
                                         stop=False)
                    nc.tensor.matmul(ps[:, 0:SGW], idt[:, :],
                                     hA[co][sg][:, :], start=False, stop=True)
                    psZ.append(ps)
                return psZ

            def do_ffn(l, sg):
                cols = slice(sg * SGW, (sg + 1) * SGW)
                Yg = []
                for co in range(4):
                    # borrow the attention-phase PSUM banks (idle in pass B)
                    ps = psum.tile([128, 512], f32,
                                   tag=("S2", "D", "Ob", "mm")[co],
                                   name="ffn1", bufs=(3 if co == 3 else 1))
                    wts = [C1[l][ci][:, co * 128:(co + 1) * 128]
                           for ci in range(4)]
                    bias_ap = (None if zero_bias else
                               BIAS[l][4:5, co * 128:(co + 1) * 128])
                    accum_mm(ps[:, 0:SGW], wts,
                             lambda ci: hB[ci][sg][:, :], bias_ap)
                    yt = zpool.tile([128, SGW], bf16, tag=f"y{co}",
                                    name=f"y{co}")
                    nc.scalar.activation(yt[:, :], ps[:, 0:SGW], AF.Gelu)
                    Yg.append(yt)
                psZ2 = []
                for co in range(4):
                    ps = psum.tile([128, 512], f32, tag="mm", name="mm", bufs=3)
                    for ci in range(4):
                        nc.tensor.matmul(ps[:, 0:SGW],
                                         C2[l][ci][:, co * 128:(co + 1) * 128],
                                         Yg[ci][:, :], start=(ci == 0),
                                         stop=False)
                    if not zero_bias:
                        nc.tensor.matmul(ps[:, 0:SGW],
                                         BIAS[l][5:6, co * 128:(co + 1) * 128],
                                         onesRow[:, 0:SGW], start=False,
                                         stop=False)
                    nc.tensor.matmul(ps[:, 0:SGW], idt[:, :],
                                     hB[co][sg][:, :], start=False, stop=True)
                    psZ2.append(ps)
                return psZ2

            # ---- layers: LN block software-pipelined across groups ------
            # Per group g, emission order is:
            #   qkv/attn/wo/evac (g), ln_stats2+rstd (g-1), ln_stats1 (g),
            #   ln_apply (g-2)
            # so the sumsq matmul for g-1 has a whole group of PE work
            # between it and the DVE chain that feeds it, and the rstd
            # DRAM roundtrip gets a full group of slack before apply.
            def ln_tail(state, ln_id, dst, aff, on_apply=None):
                """Advance the 4-stage deferred-LN pipeline one step:
                sumsq+rstd for g-1, rstd-broadcast for g-2, apply-mul for
                g-3 (records advance through the `state` deque)."""
                for rec in state:
                    rec["age"] += 1
                    if rec["age"] == 1:
                        ssb = ln_stats2(ln_id, rec["sg"], rec["sq"])
                        if KLN >= 2:
                            rec["y"] = ln_rstd(ln_id, rec["sg"], ssb)
                    elif rec["age"] == 2 and KLN >= 3:
                        rec["rB"] = ln_bcast(ln_id, rec["sg"], rec["y"])
                    elif rec["age"] == 3 and KLN >= 3:
                        ln_mul(rec["sg"], rec["zp"], rec["rB"], dst, aff)
                        if on_apply is not None:
                            on_apply(rec["sg"])
                state[:] = [r for r in state if r["age"] < 3]

            def ln_drain(state, ln_id, dst, aff, on_apply=None):
                while state:
                    ln_tail(state, ln_id, dst, aff, on_apply)

            def ln_push(state, sg, zg, tagp):
                zp = [zppool.tile([128, SGW], bf16,
                                  tag=f"zpA_{sg % 4}_{c}",
                                  name=f"{tagp}_{sg}_{c}")
                      for c in range(4)]
                sq = ln_stats1(sg, zg, zp)
                state.append({"sg": sg, "sq": sq, "zp": zp, "rB": None,
                              "age": 0})

            def ln_state():
                return []

            # ---- final LN + projection machinery (driven from inside
            # layer E-1's pass B so the drain overlaps FFN work) ----------
            lnf = 2 * E
            fin_sq, fin_pj, fin_pushed, fin_y = [], [], set(), {}

            def proj_out(sg, zpf, y):
                cols = slice(sg * SGW, (sg + 1) * SGW)
                rB = ln_bcast(lnf, sg, y)
                psf = psum.tile([128, 512], f32,
                                tag=("S2", "D", "Ob", "mm")[sg % 4],
                                name="proj", bufs=(3 if sg % 4 == 3 else 1))
                ps = psf[0:COUT, :]
                for ci in range(4):
                    nc.tensor.matmul(ps[:, 0:SGW], pjt[ci][:, :],
                                     zpf[ci][:, :],
                                     start=(ci == 0),
                                     stop=(ci == 3 and zero_bias))
                if not zero_bias:
                    nc.tensor.matmul(ps[:, 0:SGW], pjb[:, :],
                                     onesRow[:, 0:SGW],
                                     start=False, stop=True)
                osb = opool.tile([COUT, SGW], f32, tag="osb", name="osb")
                nc.vector.tensor_mul(osb[:, :], ps[0:COUT, 0:SGW],
                                     rB[0:COUT, :])
                nc.sync.dma_start(out=out_d[:, cols], in_=osb[:, :])

            def fin_push(sg):
                fin_pushed.add(sg)
                if trivial_affine:
                    # hA is an LN2 output: mean 0, var 1/(1+eps) -- the
                    # final LN is an identity to O(eps), and its affine is
                    # already folded into the projection host-side. Project
                    # straight off the residual stream.
                    cols = slice(sg * SGW, (sg + 1) * SGW)
                    psf = psum.tile([128, 512], f32,
                                    tag=("S2", "D", "Ob", "mm")[sg % 4],
                                    name="proj",
                                    bufs=(3 if sg % 4 == 3 else 1))
                    ps = psf[0:COUT, :]
                    for ci in range(4):
                        nc.tensor.matmul(ps[:, 0:SGW], pjt[ci][:, :],
                                         hA[ci][sg][:, :],
                                         start=(ci == 0), stop=(ci == 3))
                    osb = opool.tile([COUT, SGW], f32, tag="osb", name="osb")
                    cp3(sg, osb[:, :], ps[0:COUT, 0:SGW])
                    nc.sync.dma_start(out=out_d[:, cols], in_=osb[:, :])
                    return
                zpf = [zppool.tile([128, SGW], bf16,
                                   tag=f"zpF_{sg % 4}_{c}",
                                   name=f"zpf_{sg}_{c}")
                       for c in range(4)]
                sqf = ln_stats1(sg, [hA[c][sg][:, :] for c in range(4)], zpf)
                if fin_sq:
                    psg, psqf = fin_sq.pop(0)
                    pssb = ln_stats2(lnf, psg, psqf)
                    fin_y[psg] = ln_rstd(lnf, psg, pssb)
                if len(fin_pj) >= 3:
                    psg, pzpf = fin_pj.pop(0)
                    proj_out(psg, pzpf, fin_y[psg])
                fin_sq.append((sg, sqf))
                fin_pj.append((sg, zpf))

            def fin_drain():
                for psg, psqf in fin_sq:
                    pssb = ln_stats2(lnf, psg, psqf)
                    fin_y[psg] = ln_rstd(lnf, psg, pssb)
                del fin_sq[:]
                for psg, pzpf in fin_pj:
                    proj_out(psg, pzpf, fin_y[psg])
                del fin_pj[:]

            for l in range(E if KPHASE >= 2 else 0):
                ln1, ln2 = 2 * l, 2 * l + 1
                load_weights(l)
                aff1 = None if trivial_affine else AFFT[l][0]
                aff2 = None if trivial_affine else AFFT[l][1]
                # pass A: attention + LN1 for all subgroups (exp act-table)
                stA = ln_state()
                for g in range(NG):
                    Qg, Kg, Vg = do_qkv(l, g)
                    if KPHASE < 3:
                        continue
                    sg = g
                    o2 = gpool.tile([128, 4 * SGW], bf16, tag=f"o2{sg % 2}",
                                    name=f"o2{sg % 2}", bufs=1)
                    for bj in range(SGB):
                        attn_batch(Qg, Kg, Vg, bj, o2)
                    if KPHASE < 4:
                        continue
                    psZ = do_wo(l, sg, o2)
                    if KPHASE < 5:
                        continue
                    zg = evac_z(sg, psZ)
                    ln_tail(stA, ln1, hB, aff1)
                    ln_push(stA, sg, zg, f"zpA{l}")
                if KPHASE >= 5:
                    ln_drain(stA, ln1, hB, aff1)
                if KPHASE < 6:
                    continue
                # pass B: FFN + LN2 for all subgroups (gelu act-table)
                fincb = (fin_push if (l == E - 1 and KFINAL and KLN >= 3)
                         else None)
                stB = ln_state()
                for sg in range(NSG):
                    psZ2 = do_ffn(l, sg)
                    zg2 = evac_z(sg, psZ2)
                    ln_tail(stB, ln2, hA, aff2, on_apply=fincb)
                    ln_push(stB, sg, zg2, f"zpB{l}")
                ln_drain(stB, ln2, hA, aff2, on_apply=fincb)
            # end layers

            # ---- final LN + projection: drain what pass B started -------
            if KPHASE >= 2 and KFINAL:
                for sg in range(NSG):
                    if sg not in fin_pushed:
                        fin_push(sg)
                fin_drain()
            if KPHASE < 2 or not KFINAL:
                # debug: dump embed output
                for sg in range(NSG):
                    cols = slice(sg * SGW, (sg + 1) * SGW)
                    osb = opool.tile([COUT, SGW], f32, tag="osb", name="osb")
                    nc.vector.tensor_copy(osb[:, :], hA[0][sg][0:COUT, :])
                    nc.sync.dma_start(out=out_d[:, cols], in_=osb[:, :])

    nc.compile()
    return nc


# ---------------------------------------------------------------------------
# host side
# ---------------------------------------------------------------------------

def _pos_encoding():
    pos = np.arange(L)[:, None].astype(np.float32)
    div = np.exp(np.arange(0, D, 2).astype(np.float32) * (-np.log(10000.0) / D))
    pe = np.zeros((L, D), dtype=np.float32)
    pe[:, 0::2] = np.sin(pos * div)
    pe[:, 1::2] = np.cos(pos * div)
    return pe


def _chunk4(mT):
    """[D, N] -> [4, 128, N]"""
    return np.ascontiguousarray(mT.reshape(4, 128, -1))


_NC = None
_NC_FLAGS = None


def _get_nc(trivial_affine=True, zero_bias=True):
    global _NC, _NC_FLAGS
    if _NC is None or _NC_FLAGS != (trivial_affine, zero_bias):
        _NC = build_nc(trivial_affine, zero_bias)
        _NC_FLAGS = (trivial_affine, zero_bias)
    return _NC


def is_trivial_affine(inputs):
    i = {k: np.asarray(v) for k, v in inputs.items()}
    return (np.all(i["ln1s"] == 1.0) and np.all(i["ln1b"] == 0.0)
            and np.all(i["ln2s"] == 1.0) and np.all(i["ln2b"] == 0.0))


def is_zero_bias(inputs):
    i = {k: np.asarray(v) for k, v in inputs.items()}
    return all(bool(np.all(i[k] == 0.0))
               for k in ("bq", "bk", "bv", "bo", "c1b", "c2b", "proj_b",
                         "lnfb"))


def prepare_maps(inputs):
    inp = {k: np.asarray(v) for k, v in inputs.items()}
    x = inp["x"].astype(np.float32)
    emb_w = inp["emb_w"].astype(np.float32)
    mask = inp["mask"].astype(np.float32)

    scale = 1.0 / np.sqrt(DH)

    wqt = np.stack([_chunk4(inp["Wq"][l].T * scale) for l in range(E)]).astype(BF16)
    wkt = np.stack([_chunk4(inp["Wk"][l].T) for l in range(E)]).astype(BF16)
    wvt = np.stack([_chunk4(inp["Wv"][l].T) for l in range(E)]).astype(BF16)
    wot = np.stack([_chunk4(inp["Wo"][l].T) for l in range(E)]).astype(BF16)
    c1wt = np.stack([_chunk4(inp["c1w"][l].T) for l in range(E)]).astype(BF16)
    c2wt = np.stack([_chunk4(inp["c2w"][l].T) for l in range(E)]).astype(BF16)

    biasd = np.zeros((E, 8, D), np.float32)
    for l in range(E):
        biasd[l, 0] = inp["bq"][l] * scale
        biasd[l, 1] = inp["bk"][l]
        biasd[l, 2] = inp["bv"][l]
        biasd[l, 3] = inp["bo"][l]
        biasd[l, 4] = inp["c1b"][l]
        biasd[l, 5] = inp["c2b"][l]
    affd = np.zeros((E, 2, 2, 4, 128), np.float32)
    for l in range(E):
        affd[l, 0, 0] = inp["ln1s"][l].reshape(4, 128)
        affd[l, 0, 1] = inp["ln1b"][l].reshape(4, 128)
        affd[l, 1, 0] = inp["ln2s"][l].reshape(4, 128)
        affd[l, 1, 1] = inp["ln2b"][l].reshape(4, 128)

    projw_eff = inp["proj_w"] * inp["lnfs"][None, :]
    projb_eff = inp["proj_b"] + inp["lnfb"] @ inp["proj_w"].T
    projt = np.ascontiguousarray(projw_eff.T.reshape(4, 128, COUT)).astype(BF16)

    pet = np.ascontiguousarray(
        np.tile(_pos_encoding().T.reshape(4, 128, L), (1, 1, SGB))).astype(BF16)
    wcat = np.concatenate([emb_w[:, :, 0].T, emb_w[:, :, 1].T,
                           emb_w[:, :, 2].T], axis=0)
    ident = np.eye(128, dtype=np.float32).astype(BF16)

    onesdd = np.concatenate([
        np.full((128, 128), 1.0 / D, np.float32),
        np.ones((128, 32), np.float32),
        np.concatenate([np.ones((128, 64), np.float32),
                        np.zeros((128, 64), np.float32)], axis=1),
        np.concatenate([np.zeros((128, 64), np.float32),
                        np.ones((128, 64), np.float32)], axis=1)], axis=1)
    shared = dict(
        onesdd=onesdd.astype(BF16),
        wcat=wcat.astype(BF16), petd=pet, wqt=wqt, wkt=wkt, wvt=wvt, wot=wot,
        c1wt=c1wt, c2wt=c2wt, m01d=np.tile(mask.T, (1, 8)).astype(BF16),
        maskbd=np.tile(-30.0 * (1.0 - mask).T, (1, 4)).astype(BF16),
        identd=ident,
        projt=projt, biasd=biasd.astype(BF16),
        projbd=projb_eff.reshape(1, COUT).astype(BF16), affd=affd,
    )

    in_maps = []
    for ci in range(NC_CORES):
        xs = x[ci * BL:(ci + 1) * BL]                      # [32, 100, 38]
        xp = np.concatenate([xs[:, -1:], xs, xs[:, :1]], axis=1)  # [32,102,38]
        feats = [xp[:, w:w + L, :] for w in range(3)]      # each [32,100,38]
        xaug = np.concatenate(feats, axis=2)               # [32,100,114]
        xaugT = np.ascontiguousarray(
            xaug.reshape(T, KC).T).astype(BF16)            # [114, 3200]
        m = dict(shared)
        m["xaugT"] = xaugT
        in_maps.append(m)
    return in_maps


def run(inputs, **kw):
    nc = _get_nc(is_trivial_affine(inputs), is_zero_bias(inputs))
    in_maps = prepare_maps(inputs)
    res = run_bass_kernel_spmd(nc, in_maps, core_ids=list(range(NC_CORES)), **kw)
    outs = []
    for ci in range(NC_CORES):
        o = np.asarray(res.results[ci]["out"], np.float32)  # [38, 3200]
        outs.append(o.T.reshape(BL, L, COUT))
    full = np.concatenate(outs, axis=0)
    return full, res


def kernel(**inputs):
    full, _ = run(inputs)
    return full.astype(np.float32)


def bench(inputs, iters=6):
    """Steady-state wall timing of the sharded jitted executable."""
    import time
    import jax
    from jax.sharding import Mesh, PartitionSpec
    from jax.experimental.shard_map import shard_map
    from concourse import mybir
    from concourse.bass2jax import _bass_exec_p, install_neuronx_cc_hook, partition_id_tensor

    nc = _get_nc(is_trivial_affine(inputs), is_zero_bias(inputs))
    in_maps = prepare_maps(inputs)
    install_neuronx_cc_hook()
    partition_name = nc.partition_id_tensor.name if nc.partition_id_tensor else None
    in_names, out_names, out_avals, zero_outs = [], [], [], []
    for alloc in nc.m.functions[0].allocations:
        if not isinstance(alloc, mybir.MemoryLocationSet):
            continue
        name = alloc.memorylocations[0].name
        if alloc.kind == "ExternalInput":
            if name != partition_name:
                in_names.append(name)
        elif alloc.kind == "ExternalOutput":
            out_names.append(name)
            shape = tuple(alloc.tensor_shape)
            dtype = mybir.dt.np(alloc.dtype)
            out_avals.append(jax.core.ShapedArray(shape, dtype))
            zero_outs.append(np.zeros(shape, dtype))
    n_params = len(in_names)
    n_outs = len(out_avals)
    all_names = list(in_names) + out_names + ([partition_name] if partition_name else [])

    def _body(*args):
        operands = list(args)
        if partition_name is not None:
            operands.append(partition_id_tensor())
        return tuple(_bass_exec_p.bind(
            *operands, out_avals=tuple(out_avals), in_names=tuple(all_names),
            out_names=tuple(out_names), lowering_input_output_aliases=(),
            sim_require_finite=True, sim_require_nnan=True, nc=nc))

    devices = jax.devices()[:NC_CORES]
    mesh = Mesh(np.array(devices), ("core",))
    donate = tuple(range(n_params, n_params + n_outs))
    sharded = jax.jit(
        shard_map(_body, mesh=mesh,
                  in_specs=(PartitionSpec("core"),) * (n_params + n_outs),
                  out_specs=(PartitionSpec("core"),) * n_outs,
                  check_rep=False),
        donate_argnums=donate, keep_unused=True)
    concat_in = [np.concatenate([np.asarray(in_maps[c][n]) for c in range(NC_CORES)], axis=0)
                 for n in in_names]
    dev_in = [jax.device_put(a) for a in concat_in]
    times = []
    out = None
    for it in range(iters):
        zeros = [jax.device_put(np.zeros((NC_CORES * z.shape[0], *z.shape[1:]), z.dtype))
                 for z in zero_outs]
        jax.block_until_ready(zeros)
        t0 = time.perf_counter()
        out = sharded(*dev_in, *zeros)
        jax.block_until_ready(out)
        times.append(time.perf_counter() - t0)
    res = np.asarray(out[0]).reshape(NC_CORES, COUT, T)
    full = np.concatenate([res[c].T.reshape(BL, L, COUT) for c in range(NC_CORES)], axis=0)
    return full, times

